# revision 5
# baseline (speedup 1.0000x reference)
"""ComplexMultiheadAttention on 8 TRN2 NeuronCores.

Sharding: data-parallel over batch (B=2 -> 2 groups of 4 cores); within a
group, tensor-parallel over heads (16 heads -> 4 heads/core). Each core runs
the full pipeline (complex QKV projections, attention, complex out-proj) for
its (batch, head-group) slice, producing a partial y^T; the host sums the 4
partials per batch (the "all-reduce") and transposes back.

Layout tricks (all matmuls are N=512, f32r, K=128):
 - complex packing: contraction over [real(64)|imag(64)] stacked into K=128
 - scores computed transposed (S^T[j,i]) so softmax'd probs P^T feed the
   PV matmul directly as lhsT -- no on-device transposes anywhere
 - softmax denominators via a ones-column appended to V (row 64 of the
   "or" PV output accumulates sum_j P^T[j,i])
 - V bias folded in after normalization (probs sum to 1): + v_b per row
"""
import os
import numpy as np

from concourse import bacc
import concourse.mybir as mybir
import concourse.tile as tile
from concourse.bass_utils import run_bass_kernel_spmd

B, T, D, H = 2, 2048, 1024, 16
d = D // H          # 64
NCORES = 8
HPC = 4             # heads per core
NDT = D // 128      # 8  k-tiles over model dim
NIC = T // 512      # 4  i-chunks (query)
NJT = T // 128      # 16 j-tiles (key)
NET = D // 128      # 8  e-tiles (out-proj output dim)

F32R = mybir.dt.float32r
F32 = mybir.dt.float32
AF = mybir.ActivationFunctionType

_PROG = None


def _build_program():
    nc = bacc.Bacc()
    xr = nc.dram_tensor("xr", [NDT, 128, T], F32R, kind="ExternalInput")
    xi = nc.dram_tensor("xi", [NDT, 128, T], F32R, kind="ExternalInput")
    aq = nc.dram_tensor("aq", [2, HPC, NDT, 128, 128], F32R, kind="ExternalInput")
    ak = nc.dram_tensor("ak", [2, HPC, NDT, 128, 128], F32R, kind="ExternalInput")
    av = nc.dram_tensor("av", [2, NDT, 128, 512], F32R, kind="ExternalInput")
    ao = nc.dram_tensor("ao", [2, HPC, NET, 128, 128], F32R, kind="ExternalInput")
    qb = nc.dram_tensor("qb", [128, HPC], F32, kind="ExternalInput")
    kb = nc.dram_tensor("kb", [128, HPC], F32, kind="ExternalInput")
    vb = nc.dram_tensor("vb", [64, 2 * HPC], F32, kind="ExternalInput")
    ob = nc.dram_tensor("ob", [128, 2, NET], F32, kind="ExternalInput")
    yt = nc.dram_tensor("yt", [2, D, T], F32, kind="ExternalOutput")

    with tile.TileContext(nc) as tc:
        with tc.tile_pool(name="bias", bufs=1) as biasp, \
             tc.tile_pool(name="store", bufs=1) as store:
            qb_sb = biasp.tile([128, HPC], F32, tag="qb")
            kb_sb = biasp.tile([128, HPC], F32, tag="kb")
            vb_sb = biasp.tile([64, 2 * HPC], F32, tag="vb")
            ob_sb = biasp.tile([128, 2, NET], F32, tag="ob")
            nc.sync.dma_start(qb_sb[:], qb[:])
            nc.sync.dma_start(kb_sb[:], kb[:])
            nc.sync.dma_start(vb_sb[:], vb[:])
            nc.sync.dma_start(ob_sb[:], ob[:])

            # persistent per-head products
            QT = [store.tile([128, T], F32R, tag=f"qt{h}", name=f"qt{h}") for h in range(HPC)]
            KT = [store.tile([128, T], F32R, tag=f"kt{h}", name=f"kt{h}") for h in range(HPC)]
            # V per j-tile: per head cols [vr(64) | ones(1) | vi(64)] = 129
            VS = [store.tile([128, HPC, 129], F32R, tag=f"v{jt}", name=f"v{jt}") for jt in range(NJT)]
            OT = [store.tile([128, T], F32R, tag=f"ot{h}", name=f"ot{h}") for h in range(HPC)]

            # ---------------- Phase 1: Q & K projections (fused x pass) -----
            with tc.tile_pool(name="wqk", bufs=1) as wqk, \
                 tc.tile_pool(name="xs1", bufs=3) as xs1, \
                 tc.tile_pool(name="psqk", bufs=1, space="PSUM") as psqk:
                aq_sb = wqk.tile([128, 2, HPC, NDT, 128], F32R, tag="aq")
                ak_sb = wqk.tile([128, 2, HPC, NDT, 128], F32R, tag="ak")
                for j in range(2):
                    for h in range(HPC):
                        for dt in range(NDT):
                            nc.sync.dma_start(aq_sb[:, j, h, dt, :], aq[j, h, dt])
                            nc.sync.dma_start(ak_sb[:, j, h, dt, :], ak[j, h, dt])
                for ic in range(NIC):
                    icsl = slice(ic * 512, (ic + 1) * 512)
                    psq = [psqk.tile([128, 512], F32, tag=f"psq{h}", name=f"psq{h}") for h in range(HPC)]
                    psk = [psqk.tile([128, 512], F32, tag=f"psk{h}", name=f"psk{h}") for h in range(HPC)]
                    for dt in range(NDT):
                        xrt = xs1.tile([128, 512], F32R, tag="xr")
                        xit = xs1.tile([128, 512], F32R, tag="xi")
                        nc.sync.dma_start(xrt[:], xr[dt][:, icsl])
                        nc.sync.dma_start(xit[:], xi[dt][:, icsl])
                        for h in range(HPC):
                            st = (dt == 0)
                            sp = (dt == NDT - 1)
                            nc.tensor.matmul(psq[h][:], aq_sb[:, 0, h, dt, :], xrt[:], start=st, stop=False)
                            nc.tensor.matmul(psq[h][:], aq_sb[:, 1, h, dt, :], xit[:], start=False, stop=sp)
                            nc.tensor.matmul(psk[h][:], ak_sb[:, 0, h, dt, :], xrt[:], start=st, stop=False)
                            nc.tensor.matmul(psk[h][:], ak_sb[:, 1, h, dt, :], xit[:], start=False, stop=sp)
                    for h in range(HPC):
                        nc.scalar.activation(QT[h][:, icsl], psq[h][:], AF.Identity,
                                             bias=qb_sb[:, h:h + 1])
                        nc.scalar.activation(KT[h][:, icsl], psk[h][:], AF.Identity,
                                             bias=kb_sb[:, h:h + 1])

            # ---------------- Phase 2: V projection ------------------------
            with tc.tile_pool(name="wv", bufs=1) as wv, \
                 tc.tile_pool(name="xs2", bufs=3) as xs2, \
                 tc.tile_pool(name="psv", bufs=2, space="PSUM") as psv:
                av_sb = wv.tile([128, 2, NDT, 512], F32R, tag="av")
                for j in range(2):
                    for dt in range(NDT):
                        nc.sync.dma_start(av_sb[:, j, dt, :], av[j, dt])
                for ic in range(NIC):
                    icsl = slice(ic * 512, (ic + 1) * 512)
                    pv = [psv.tile([128, 512], F32, tag=f"pv{jj}", name=f"pv{jj}") for jj in range(4)]
                    xrts, xits = [], []
                    for dt in range(NDT):
                        xrt = xs2.tile([128, 512], F32R, tag="xr2")
                        xit = xs2.tile([128, 512], F32R, tag="xi2")
                        nc.sync.dma_start(xrt[:], xr[dt][:, icsl])
                        nc.sync.dma_start(xit[:], xi[dt][:, icsl])
                        for jj in range(4):
                            jsl = slice(jj * 128, (jj + 1) * 128)
                            nc.tensor.matmul(pv[jj][:], xrt[:, jsl], av_sb[:, 0, dt, :],
                                             start=(dt == 0), stop=False)
                            nc.tensor.matmul(pv[jj][:], xit[:, jsl], av_sb[:, 1, dt, :],
                                             start=False, stop=(dt == NDT - 1))
                    for jj in range(4):
                        jt = ic * 4 + jj
                        nc.vector.memset(VS[jt][:, :, 64:65].bitcast(F32), 1.0)
                        for h in range(HPC):
                            nc.scalar.activation(VS[jt][:, h, 0:64], pv[jj][:, h * 128:h * 128 + 64], AF.Copy)
                            nc.scalar.activation(VS[jt][:, h, 65:129], pv[jj][:, h * 128 + 64:h * 128 + 128], AF.Copy)

            # ---------------- Phase 3: attention ---------------------------
            with tc.tile_pool(name="pexp", bufs=3) as pexp, \
                 tc.tile_pool(name="pnorm", bufs=2) as pnorm, \
                 tc.tile_pool(name="pss", bufs=2, space="PSUM") as pss, \
                 tc.tile_pool(name="pso", bufs=2, space="PSUM") as pso:
                for h in range(HPC):
                    for ic in range(NIC):
                        icsl = slice(ic * 512, (ic + 1) * 512)
                        ps_or = pso.tile([65, 512], F32, tag="por")
                        ps_oi = pso.tile([64, 512], F32, tag="poi")
                        for jt in range(NJT):
                            jsl = slice(jt * 128, (jt + 1) * 128)
                            ps_s = pss.tile([128, 512], F32, tag="s")
                            nc.tensor.matmul(ps_s[:], KT[h][:, jsl], QT[h][:, icsl],
                                             start=True, stop=True)
                            pt = pexp.tile([128, 512], F32R, tag="pt")
                            nc.scalar.activation(pt[:], ps_s[:], AF.Exp, scale=0.125)
                            nc.tensor.matmul(ps_or[:], VS[jt][:, h, 0:65], pt[:],
                                             start=(jt == 0), stop=(jt == NJT - 1))
                            nc.tensor.matmul(ps_oi[:], VS[jt][:, h, 65:129], pt[:],
                                             start=(jt == 0), stop=(jt == NJT - 1))
                        recip = pnorm.tile([1, 512], F32, tag="recip")
                        nc.vector.reciprocal(recip[:], ps_or[64:65, :])
                        rbc = pnorm.tile([64, 512], F32, tag="rbc")
                        nc.gpsimd.partition_broadcast(rbc[:], recip[:], channels=64)
                        # or rows -> OT[h][0:64] directly (same partition base)
                        tmp_r = pnorm.tile([64, 512], F32, tag="tr")
                        nc.vector.tensor_mul(tmp_r[:], ps_or[0:64, :], rbc[:])
                        nc.vector.tensor_add(OT[h][0:64, icsl], tmp_r[:],
                                             vb_sb[:, 2 * h:2 * h + 1].to_broadcast((64, 512)))
                        # oi rows -> OT[h][64:128] via DMA (partition shift)
                        tmp_i = pnorm.tile([64, 512], F32R, tag="ti")
                        nc.vector.tensor_mul(tmp_i[:], ps_oi[0:64, :], rbc[:])
                        nc.vector.tensor_add(tmp_i[:], tmp_i[:],
                                             vb_sb[:, 2 * h + 1:2 * h + 2].to_broadcast((64, 512)))
                        nc.sync.dma_start(OT[h][64:128, icsl], tmp_i[:])

            # ---------------- Phase 4: out projection ----------------------
            with tc.tile_pool(name="wo", bufs=1) as wo, \
                 tc.tile_pool(name="ys", bufs=3) as ys, \
                 tc.tile_pool(name="psy", bufs=2, space="PSUM") as psy:
                ao_sb = wo.tile([128, 2, HPC, NET, 128], F32R, tag="ao")
                for j in range(2):
                    for h in range(HPC):
                        for et in range(NET):
                            nc.sync.dma_start(ao_sb[:, j, h, et, :], ao[j, h, et])
                for et in range(NET):
                    esl = slice(et * 128, (et + 1) * 128)
                    for ic in range(NIC):
                        icsl = slice(ic * 512, (ic + 1) * 512)
                        ps_yr = psy.tile([128, 512], F32, tag="yr")
                        ps_yi = psy.tile([128, 512], F32, tag="yi")
                        for h in range(HPC):
                            nc.tensor.matmul(ps_yr[:], ao_sb[:, 0, h, et, :], OT[h][:, icsl],
                                             start=(h == 0), stop=(h == HPC - 1))
                            nc.tensor.matmul(ps_yi[:], ao_sb[:, 1, h, et, :], OT[h][:, icsl],
                                             start=(h == 0), stop=(h == HPC - 1))
                        ytr = ys.tile([128, 512], F32, tag="ytr")
                        yti = ys.tile([128, 512], F32, tag="yti")
                        nc.scalar.activation(ytr[:], ps_yr[:], AF.Identity,
                                             bias=ob_sb[:, 0, et:et + 1])
                        nc.scalar.activation(yti[:], ps_yi[:], AF.Identity,
                                             bias=ob_sb[:, 1, et:et + 1])
                        nc.sync.dma_start(yt[0, esl, icsl], ytr[:])
                        nc.sync.dma_start(yt[1, esl, icsl], yti[:])

    nc.finalize()
    return nc


def _prep_core(inp, b, g):
    """Build the per-core input dict for batch b, head-group g."""
    hs = slice(g * HPC * d, (g + 1) * HPC * d)          # 256 cols of D
    ch = [slice((g * HPC + hh) * d, (g * HPC + hh + 1) * d) for hh in range(HPC)]

    xrT = np.ascontiguousarray(inp["x_real"][b].T)       # [D, T]
    xiT = np.ascontiguousarray(inp["x_imag"][b].T)

    def qk_stack(wr, wi):
        a = np.empty((2, HPC, NDT, 128, 128), np.float32)
        for hh in range(HPC):
            wrh, wih = wr[:, ch[hh]], wi[:, ch[hh]]      # [D, 64]
            a[0, hh] = np.concatenate([wrh, wih], axis=1).reshape(NDT, 128, 128)
            a[1, hh] = np.concatenate([-wih, wrh], axis=1).reshape(NDT, 128, 128)
        return a

    aq = qk_stack(inp["q_wr"], inp["q_wi"])
    ak = qk_stack(inp["k_wr"], inp["k_wi"])

    av = np.empty((2, NDT, 128, 512), np.float32)
    vr_w, vi_w = inp["v_wr"], inp["v_wi"]
    av0 = np.concatenate([np.concatenate([vr_w[:, ch[hh]], vi_w[:, ch[hh]]], axis=1)
                          for hh in range(HPC)], axis=1)  # [D, 512]
    av1 = np.concatenate([np.concatenate([-vi_w[:, ch[hh]], vr_w[:, ch[hh]]], axis=1)
                          for hh in range(HPC)], axis=1)
    av[0] = av0.reshape(NDT, 128, 512)
    av[1] = av1.reshape(NDT, 128, 512)

    ao = np.empty((2, HPC, NET, 128, 128), np.float32)
    owr, owi = inp["o_wr"], inp["o_wi"]
    for hh in range(HPC):
        r0 = np.concatenate([owr[ch[hh], :], -owi[ch[hh], :]], axis=0)   # [128, D]
        r1 = np.concatenate([owi[ch[hh], :], owr[ch[hh], :]], axis=0)
        ao[0, hh] = r0.reshape(128, NET, 128).transpose(1, 0, 2)
        ao[1, hh] = r1.reshape(128, NET, 128).transpose(1, 0, 2)

    def bias_stack(br, bi):
        out = np.empty((128, HPC), np.float32)
        for hh in range(HPC):
            out[0:64, hh] = br[ch[hh]]
            out[64:128, hh] = bi[ch[hh]]
        return out

    vb_host = np.empty((64, 2 * HPC), np.float32)
    for hh in range(HPC):
        vb_host[:, 2 * hh] = inp["v_br"][ch[hh]]
        vb_host[:, 2 * hh + 1] = inp["v_bi"][ch[hh]]

    ob = np.zeros((128, 2, NET), np.float32)
    if g == 0:  # out-proj bias added on exactly one core per batch group
        ob[:, 0, :] = inp["o_br"].reshape(NET, 128).T
        ob[:, 1, :] = inp["o_bi"].reshape(NET, 128).T

    return {
        "xr": xrT.reshape(NDT, 128, T), "xi": xiT.reshape(NDT, 128, T),
        "aq": aq, "ak": ak, "av": av, "ao": ao,
        "qb": bias_stack(inp["q_br"], inp["q_bi"]),
        "kb": bias_stack(inp["k_br"], inp["k_bi"]),
        "vb": vb_host,
        "ob": ob,
    }


def kernel(**inputs):
    global _PROG
    inp = {k: np.asarray(v, np.float32) for k, v in inputs.items()}
    if _PROG is None:
        _PROG = _build_program()
    in_maps = [_prep_core(inp, c // 4, c % 4) for c in range(NCORES)]
    trace = os.environ.get("KBENCH_TRACE") == "1"
    import time as _time
    t0 = _time.time()
    res = run_bass_kernel_spmd(_PROG, in_maps, core_ids=list(range(NCORES)),
                               trace=trace)
    kernel.last_run_wall_ns = int((_time.time() - t0) * 1e9)
    if trace:
        kernel.last_exec_time_ns = res.exec_time_ns
        kernel.last_trace = res.instructions_and_trace
    y = np.zeros((2, B, T, D), np.float32)
    for c in range(NCORES):
        b = c // 4
        ytc = res.results[c]["yt"]          # [2, D, T]
        y[0, b] += ytc[0].T
        y[1, b] += ytc[1].T
    return y
